# revision 1
# baseline (speedup 1.0000x reference)
"""GQA (32 q heads / 8 kv heads, RoPE, causal) Trainium2 Bass kernel.

Sharding: tensor-parallel over kv heads — core c owns kv head c and q heads
4c..4c+3 for both batches. Each core computes a partial o-projection
(its 256 attn channels x Wo columns) and the host sums the 8 partials.

Device-side structure (per core, per batch):
  * Fused QKV projection: one accumulation chain per 128-token tile produces
    [t, 384] = [4 q heads | k head | v head] with d contracted on partitions
    (host passes x pre-transposed).  float32r matmuls (1 cycle/row).
  * RoPE applied in token-partition layout with stride-2 free-dim APs
    (interleaved even/odd pairs), 6 DVE ops per tile covering all 5 heads.
  * Q/K transposed per-head via TensorE into [dh, t] (f32r), V kept natural
    [t, dh] with a ones column appended.
  * Scores computed transposed [keys, queries]; exp on ACT (no max needed:
    |scores| small by construction); causal diagonal masked by DVE multiply.
  * attn.V matmul gives attnT [dh, i] plus the softmax denominator for free
    (ones row of V); normalization via reciprocal + PE broadcast + DVE mul
    writes attnT directly into the o-projection's stationary layout [c, t].
"""

import numpy as np
from contextlib import ExitStack

import concourse.bass as bass
from concourse import bacc
import concourse.mybir as mybir
import concourse.tile as tile
from concourse.bass_utils import run_bass_kernel_spmd

B, S, D = 2, 2048, 2048
DH = 64            # head dim
G = 4              # q heads per core (= per kv head)
NCORES = 8
TT = 512           # attention i-tile
NTT = S // TT      # 4
KC = D // 128      # 16 contraction chunks
NJC = S // 128     # 16 token/key chunks of 128
F32 = mybir.dt.float32
F32R = mybir.dt.float32r
ROPE_BASE = 10000.0

_cached = {}


def build_nc():
    nc = bacc.Bacc("TRN2", target_bir_lowering=False, debug=False)
    xt = nc.declare_dram_parameter("xt", [B, D, S], F32, isOutput=False)
    wall = nc.declare_dram_parameter("wall", [D, 384], F32, isOutput=False)
    wot = nc.declare_dram_parameter("wot", [256, D], F32, isOutput=False)
    cosr = nc.declare_dram_parameter("cosr", [S, 160], F32, isOutput=False)
    sinr = nc.declare_dram_parameter("sinr", [S, 160], F32, isOutput=False)
    cmask = nc.declare_dram_parameter("cmask", [4, 128, TT], F32, isOutput=False)
    ident = nc.declare_dram_parameter("ident", [128, 128], F32, isOutput=False)
    o = nc.declare_dram_parameter("o", [B, S, D], F32, isOutput=True)

    EXP = mybir.ActivationFunctionType.Exp

    with tile.TileContext(nc) as tc, ExitStack() as ctx:
        wpool = ctx.enter_context(tc.tile_pool(name="weights", bufs=1))
        per_b = ctx.enter_context(tc.tile_pool(name="per_b", bufs=1))
        xpool = ctx.enter_context(tc.tile_pool(name="xstream", bufs=12))
        qkvpool = ctx.enter_context(tc.tile_pool(name="qkv", bufs=3))
        epool = ctx.enter_context(tc.tile_pool(name="exp", bufs=6))
        rpool = ctx.enter_context(tc.tile_pool(name="rope", bufs=2))
        opool = ctx.enter_context(tc.tile_pool(name="out", bufs=4))
        spool = ctx.enter_context(tc.tile_pool(name="small", bufs=4))
        pp_proj = ctx.enter_context(tc.tile_pool(name="pproj", bufs=1, space="PSUM"))
        pp_att = ctx.enter_context(tc.tile_pool(name="patt", bufs=2, space="PSUM"))
        pp_av = ctx.enter_context(tc.tile_pool(name="pav", bufs=1, space="PSUM"))
        pp_misc = ctx.enter_context(tc.tile_pool(name="pmisc", bufs=1, space="PSUM"))

        # ---- persistent weights/tables ----
        wall_sb = wpool.tile([128, KC, 384], F32R, tag="wall")
        wot_sb = wpool.tile([128, 2, D], F32R, tag="wot")
        cos_sb = wpool.tile([128, NJC, 160], F32, tag="cos")
        sin_sb = wpool.tile([128, NJC, 160], F32, tag="sin")
        mask_sb = wpool.tile([128, 4, TT], F32R, tag="mask")
        ident_sb = wpool.tile([128, 128], F32, tag="ident")
        ones_sb = wpool.tile([1, 64], F32R, tag="ones")
        for k in range(KC):
            nc.sync.dma_start(wall_sb[:, k, :],
                              wall[k * 128:(k + 1) * 128, :].bitcast(F32R))
        for cc in range(2):
            nc.sync.dma_start(wot_sb[:, cc, :],
                              wot[cc * 128:(cc + 1) * 128, :].bitcast(F32R))
        for j in range(NJC):
            nc.sync.dma_start(cos_sb[:, j, :], cosr[j * 128:(j + 1) * 128, :])
            nc.sync.dma_start(sin_sb[:, j, :], sinr[j * 128:(j + 1) * 128, :])
        for m in range(4):
            nc.sync.dma_start(mask_sb[:, m, :], cmask[m].bitcast(F32R))
        nc.sync.dma_start(ident_sb[:], ident[:, :])
        nc.vector.memset(ones_sb[:].bitcast(F32), 1.0)

        for b in range(B):
            qt = per_b.tile([64, G, S], F32R, tag="qt")
            kt = per_b.tile([64, S], F32R, tag="kt")
            vsb = per_b.tile([128, NJC, DH + 1], F32R, tag="vsb")
            at = per_b.tile([128, 2, S], F32R, tag="at")
            nc.vector.memset(vsb[:].bitcast(F32), 1.0)

            # ---------- fused QKV projection + rope + transposes ----------
            # Transposes for tile tt are emitted after tile tt+1's matmuls so
            # the PE never waits on the ACT-evict -> DVE-rope chain.
            def emit_tail(tt, qkv):
                tsl = slice(tt * 128, (tt + 1) * 128)
                for h in range(5):
                    ptr = pp_misc.tile([64, 128], F32, tag="misc")
                    nc.tensor.transpose(ptr[:], qkv[:, h * 64:(h + 1) * 64],
                                        ident_sb[:, :])
                    if h < G:
                        nc.vector.tensor_copy(qt[:, h, tsl], ptr[:])
                    else:
                        nc.vector.tensor_copy(kt[:, tsl], ptr[:])
                nc.vector.tensor_copy(vsb[:, tt, 0:DH], qkv[:, 320:384])

            prev = None
            for tg in range(4):             # groups of 512 tokens, 4 psum accs
                pq = [pp_proj.tile([128, 384], F32, tag=f"pq{s}",
                                   name=f"pq{s}_{b}_{tg}")
                      for s in range(4)]
                for k in range(KC):
                    xbig = xpool.tile([128, 512], F32R, tag="xt")
                    nc.sync.dma_start(
                        xbig[:],
                        xt[b, k * 128:(k + 1) * 128,
                           tg * 512:(tg + 1) * 512].bitcast(F32R))
                    for s in range(4):
                        nc.tensor.matmul(pq[s][:],
                                         xbig[:, s * 128:(s + 1) * 128],
                                         wall_sb[:, k, :],
                                         start=(k == 0), stop=(k == KC - 1))
                for s in range(4):
                    tt = tg * 4 + s
                    qkv = qkvpool.tile([128, 384], F32, tag="qkv")
                    nc.scalar.copy(qkv[:], pq[s][:])
                    # rope on q+k (cols 0:320), interleaved pairs in free dim
                    pear = qkv[:, 0:320].rearrange("p (h i two) -> p h i two",
                                                   two=2, i=32)
                    ev, od = pear[:, :, :, 0], pear[:, :, :, 1]
                    cs = cos_sb[:, tt, :].rearrange("p (h i) -> p h i", i=32)
                    sn = sin_sb[:, tt, :].rearrange("p (h i) -> p h i", i=32)
                    ec = rpool.tile([128, 5, 32], F32, tag="ec")
                    es = rpool.tile([128, 5, 32], F32, tag="es")
                    oc = rpool.tile([128, 5, 32], F32, tag="oc")
                    os_ = rpool.tile([128, 5, 32], F32, tag="os")
                    nc.vector.tensor_mul(ec[:], ev, cs)
                    nc.vector.tensor_mul(es[:], ev, sn)
                    nc.vector.tensor_mul(oc[:], od, cs)
                    nc.vector.tensor_mul(os_[:], od, sn)
                    nc.vector.tensor_sub(ev, ec[:], os_[:])
                    nc.vector.tensor_add(od, es[:], oc[:])
                    if prev is not None:
                        emit_tail(*prev)
                    prev = (tt, qkv)
            emit_tail(*prev)

            # ---------- attention ----------
            for g in range(G):
                cc, r0 = g // 2, (g % 2) * 64
                for it in range(NTT):
                    isl = slice(it * TT, (it + 1) * TT)
                    pav = pp_av.tile([65, TT], F32, tag="av")
                    njc = 4 * it + 4
                    pending = []  # attn.V pipelined two steps behind scores
                    for jc in range(njc):
                        psc = pp_att.tile([128, TT], F32, tag="sc")
                        nc.tensor.matmul(
                            psc[:], kt[:, jc * 128:(jc + 1) * 128],
                            qt[:, g, isl], start=True, stop=True)
                        esb = epool.tile([128, TT], F32R, tag="exp")
                        nc.scalar.activation(esb[:], psc[:], EXP, scale=0.125)
                        if jc >= 4 * it:  # diagonal block: causal mask
                            nc.vector.tensor_mul(esb[:], esb[:],
                                                 mask_sb[:, jc - 4 * it, :])
                        pending.append(((pav[:], vsb[:, jc, :], esb[:]),
                                        dict(start=(jc == 0),
                                             stop=(jc == njc - 1))))
                        if len(pending) > 2:
                            a = pending.pop(0)
                            nc.tensor.matmul(*a[0], **a[1])
                    for a in pending:
                        nc.tensor.matmul(*a[0], **a[1])
                    # normalize via ones-row sum: recip -> PE broadcast -> mul
                    rcp = spool.tile([1, TT], F32, tag="rcp")
                    nc.vector.reciprocal(rcp[:], pav[64:65, :])
                    avs = spool.tile([64, TT], F32, tag="avs")
                    nc.scalar.copy(avs[:], pav[0:64, :])
                    rcpr = spool.tile([1, TT], F32R, tag="rcpr")
                    nc.vector.tensor_copy(rcpr[:], rcp[:])
                    pbc = pp_misc.tile([64, TT], F32, tag="misc")
                    nc.tensor.matmul(pbc[:], ones_sb[:], rcpr[:],
                                     start=True, stop=True)
                    nc.vector.tensor_mul(at[r0:r0 + 64, cc, isl],
                                         avs[:], pbc[:])

            # ---------- o projection (partial over this core's channels) ----
            for tt in range(NJC):
                tsl = slice(tt * 128, (tt + 1) * 128)
                for nt in range(D // TT):
                    nsl = slice(nt * TT, (nt + 1) * TT)
                    po = pp_proj.tile([128, TT], F32, tag=f"pq{nt}",
                                      name=f"po{b}_{tt}_{nt}")
                    nc.tensor.matmul(po[:], at[:, 0, tsl], wot_sb[:, 0, nsl],
                                     start=True, stop=False)
                    nc.tensor.matmul(po[:], at[:, 1, tsl], wot_sb[:, 1, nsl],
                                     start=False, stop=True)
                    osb = opool.tile([128, TT], F32, tag="osb")
                    nc.vector.tensor_copy(osb[:], po[:])
                    nc.sync.dma_start(o[b, tsl, nsl], osb[:])
    nc.compile()
    return nc


def host_inputs(x, Wq, Wk, Wv, Wo):
    """Per-core input maps. Q/K weight rows permuted so each head is
    [interleaved] kept natural; rope works on interleaved pairs in the
    free dim, so NO permutation is needed here."""
    xtp = np.ascontiguousarray(np.transpose(np.asarray(x, np.float32), (0, 2, 1)))
    inv = ROPE_BASE ** (-np.arange(0, DH, 2, dtype=np.float64) / DH)
    th = np.arange(S, dtype=np.float64)[:, None] * inv[None, :]  # (S, 32)
    cosr = np.tile(np.cos(th), (1, 5)).astype(np.float32)  # (S, 160)
    sinr = np.tile(np.sin(th), (1, 5)).astype(np.float32)
    p = np.arange(128)[:, None]
    f = np.arange(TT)[None, :]
    cmask = np.stack([(p + m * 128 <= f).astype(np.float32) for m in range(4)])
    ident = np.eye(128, dtype=np.float32)
    in_maps = []
    for c in range(NCORES):
        wall = np.concatenate([Wq[256 * c:256 * (c + 1)],
                               Wk[DH * c:DH * (c + 1)],
                               Wv[DH * c:DH * (c + 1)]], axis=0)
        wall = np.ascontiguousarray(wall.T.astype(np.float32))       # (D, 384)
        wot = np.ascontiguousarray(Wo[:, 256 * c:256 * (c + 1)].T
                                   .astype(np.float32))              # (256, D)
        in_maps.append(dict(xt=xtp, wall=wall, wot=wot, cosr=cosr,
                            sinr=sinr, cmask=cmask, ident=ident))
    return in_maps


def kernel(**inputs):
    x = np.asarray(inputs["x"], dtype=np.float32)
    Wq = np.asarray(inputs["Wq"], dtype=np.float32)
    Wk = np.asarray(inputs["Wk"], dtype=np.float32)
    Wv = np.asarray(inputs["Wv"], dtype=np.float32)
    Wo = np.asarray(inputs["Wo"], dtype=np.float32)
    in_maps = host_inputs(x, Wq, Wk, Wv, Wo)
    if "nc" not in _cached:
        _cached["nc"] = build_nc()
    res = run_bass_kernel_spmd(_cached["nc"], in_maps, list(range(NCORES)))
    out = np.zeros((B, S, D), np.float64)
    for r in res.results:
        out += r["o"]
    return out.astype(np.float32)

